# revision 25
# baseline (speedup 1.0000x reference)
"""Sinkhorn AssignmentLoss kernel for 8 TRN2 NeuronCores.

Math: the reference's stabilized log-space Sinkhorn is equivalent (exactly,
up to fp rounding) to exp-space Sinkhorn on the positive kernel matrix
  K2 = [exp(logits - g), rowsum(exp(logits - g)) * exp(d - g)]   # [N, C+1]
with per-sample scalar g = max(max(logits), d) (scale invariance lets us drop
the softmax row-normalization into u):
  u = mu / (K2 v);  v = nu / (K2^T u);  P = diag(u) K2 diag(v)
With TEMP=1 the iteration converges in <4 iterations (measured ~5e-4 rel err
vs the reference's 20 iterations at ITERS=3, fp16 kernel storage).

Per core: 8 samples, data-parallel over batch (no collectives).
Samples run in two interleaved groups of four; DVE division work is batched
over sample pairs and software-pipelined against the other pair's PE matvecs
so the PE stream stays dense.

Device pipeline per sample:
  DMA logits -> ACT exp(+rowsum accum) -> fp16 KN [n-part, c-free],
    zero-padded to 640 cols so every weight chunk is 128 wide (FWL)
  PE transpose -> fp16 KT [c-part, n-free]
  ITERS x weights-form matvecs: K chunks are PE weights (fp16 FWL),
    u/v column vectors are the 1-wide moving operand, so matvec results
    land as PSUM columns and reciprocal_approx_fast + multiply run on all
    128 DVE lanes for four samples per instruction pair.
  P = KN * u[n] * v[c] via fp16 scalar_tensor_tensor (2x mode) -> DMA out
  (fp16 output; host upcasts to fp32 — errors stay ~1e-4 of max|P|)
"""

import sys
import numpy as np

for _p in ("/opt/trn_rl_repo", "/root/.axon_site/_ro/trn_rl_repo"):
    if _p not in sys.path:
        sys.path.insert(0, _p)

from contextlib import ExitStack

import concourse.bass as bass
import concourse.tile as tile
from concourse import bacc, mybir
from concourse.bass_utils import run_bass_kernel_spmd

B, N, C = 64, 1024, 558
CP1 = C + 1
CPAD = 640               # KN free size: 5 chunks of 128
NCORES = 8
S = B // NCORES          # samples per core
NT = N // 128            # 8 row tiles
W4 = CP1 - 512           # 47: logical width of the last c-chunk
ITERS = 3
GRP = 4                  # samples interleaved per group
MU_SCALE = 256.0         # keeps u, v in fp16 normal range; cancels exactly in P

F32 = mybir.dt.float32
F16 = mybir.dt.float16
EXP = mybir.ActivationFunctionType.Exp
MULT = mybir.AluOpType.mult


def _ap2(t, part, off, step, cnt, inner):
    """AP with partitions [0:part], free dims [[step, cnt], [1, inner]]."""
    a = t[:]
    base = list(a.ap)
    return bass.AP(
        tensor=a.tensor,
        offset=a.offset + off * base[-1][0],
        ap=[[base[0][0], part], [step * base[-1][0], cnt], [base[-1][0], inner]],
    )


def _emit_kv(nc, pools, kt, vq, k):
    """pu[:, 8k+t] += KT_j^T v_j for one sample (weights-form)."""
    pu = pools["pu"]
    for t in range(NT):
        for j in range(5):
            nc.tensor.matmul(
                pu[:, 8 * k + t : 8 * k + t + 1],
                lhsT=kt[:, j, 128 * t : 128 * (t + 1)],
                rhs=vq[:, 5 * k + j : 5 * k + j + 1],
                start=(j == 0), stop=(j == 4),
            )


def _emit_ktu(nc, pools, kn, uq, k):
    pv = pools["pv"]
    for j in range(5):
        for t in range(NT):
            nc.tensor.matmul(
                pv[:, 5 * k + j : 5 * k + j + 1],
                lhsT=kn[:, t, 128 * j : 128 * (j + 1)],
                rhs=uq[:, 8 * k + t : 8 * k + t + 1],
                start=(t == 0), stop=(t == NT - 1),
            )


def _build_kernel(ctx: ExitStack, tc: "tile.TileContext", out, lg, mu, gneg, edg, ident):
    nc = tc.nc

    pools = {
        "singles": ctx.enter_context(tc.tile_pool(name="singles", bufs=1)),
        "lgp": ctx.enter_context(tc.tile_pool(name="lgp", bufs=4)),
        "knp": ctx.enter_context(tc.tile_pool(name="knp", bufs=6)),
        "ktp": ctx.enter_context(tc.tile_pool(name="ktp", bufs=6)),
        "vecp": ctx.enter_context(tc.tile_pool(name="vecp", bufs=3)),
        "outp": ctx.enter_context(tc.tile_pool(name="outp", bufs=4)),
        "ptp": ctx.enter_context(tc.tile_pool(name="ptp", bufs=2, space="PSUM")),
        "accp": ctx.enter_context(tc.tile_pool(name="accp", bufs=4, space="PSUM")),
        "prp": ctx.enter_context(tc.tile_pool(name="prp", bufs=2, space="PSUM")),
    }
    singles = pools["singles"]

    sb_ident = singles.tile([128, 128], F16)
    nc.sync.dma_start(sb_ident[:], ident)
    sb_gneg = singles.tile([128, S], F32)
    nc.sync.dma_start(sb_gneg[:], gneg)
    sb_edg = singles.tile([128, S], F32)
    nc.sync.dma_start(sb_edg[:], edg)
    # mu in column layout: mucol[p, s, t] = MU_SCALE * mask/nv at row 128*t+p
    sb_mu = singles.tile([128, S, NT], F32)
    nc.sync.dma_start(sb_mu[:], mu)
    # broadcast weights carry 1/MU_SCALE so P = kn * u' * v'/SC
    sb_ones128 = singles.tile([1, 128], F16)
    nc.vector.memset(sb_ones128[:], 1.0 / MU_SCALE)

    def build_pair(p):
        """load + exp + transpose for samples 2p, 2p+1 -> [(kn, kt), (kn, kt)]."""
        res = []
        for s in (2 * p, 2 * p + 1):
            h0 = pools["lgp"].tile([128, 4, C], F32, tag="lgt")
            nc.sync.dma_start(
                h0[:], lg[s, 0:512].rearrange("(t p) c -> p t c", p=128)
            )
            h1 = pools["lgp"].tile([128, 4, C], F32, tag="lgt")
            nc.sync.dma_start(
                h1[:], lg[s, 512:1024].rearrange("(t p) c -> p t c", p=128)
            )
            kn = pools["knp"].tile([128, NT, CPAD], F16, tag="kn")
            sacc = pools["vecp"].tile([128, NT], F32, tag="sacc")
            nc.gpsimd.memset(kn[:, :, CP1:CPAD], 0.0)
            for t in range(NT):
                src = h0 if t < 4 else h1
                nc.scalar.activation(
                    kn[:, t, 0:C], src[:, t % 4, :], EXP,
                    bias=sb_gneg[:, s : s + 1], scale=1.0,
                    accum_out=sacc[:, t : t + 1],
                )
            nc.vector.tensor_scalar(
                kn[:, :, C], sacc[:], sb_edg[:, s : s + 1], None, MULT
            )
            kt = pools["ktp"].tile([128, 5, N], F16, tag="kt")
            for j in range(5):
                pt = pools["ptp"].tile([128, N], F16, tag="pt")
                for t in range(NT):
                    nc.tensor.transpose(
                        pt[:, 128 * t : 128 * (t + 1)],
                        kn[:, t, 128 * j : 128 * (j + 1)],
                        sb_ident[:],
                    )
                if (s + j) % 2 == 0:
                    nc.scalar.copy(kt[:, j, :], pt[:])
                else:
                    nc.vector.tensor_copy(kt[:, j, :], pt[:])
            res.append((kn, kt))
        return res

    def iter_quad(s0, built):
        """ITERS Sinkhorn iterations for samples s0..s0+3, interleaved.
        built = [(kn, kt)] * 4. Returns (uq, vq) column tiles."""
        kns = [b[0] for b in built]
        kts = [b[1] for b in built]
        vq = pools["vecp"].tile([128, 20], F16, tag="vq")
        nc.vector.memset(vq[:], 1.0)
        nc.vector.memset(_ap2(vq, 128, 4, 5, 4, 1), 0.0)
        nc.vector.memset(_ap2(vq, W4, 4, 5, 4, 1), 1.0)
        uq = None

        def u_half(pu, uq, half, uqf=None):
            o = 16 * half
            wu = pools["vecp"].tile([128, 16], F32, tag="wu")
            nc.vector.reciprocal_approx_fast(wu[:], pu[:, o : o + 16])
            a = s0 + 2 * half
            mu_sl = sb_mu[:, a : a + 2, :].rearrange("p s t -> p (s t)")
            nc.vector.tensor_mul(uq[:, o : o + 16], mu_sl, wu[:])
            if uqf is not None:
                nc.vector.tensor_mul(uqf[:, o : o + 16], mu_sl, wu[:])

        def v_half(pv, vq_new, half):
            o = 10 * half
            wv = pools["vecp"].tile([128, 10], F32, tag="wv")
            nc.vector.reciprocal_approx_fast(
                _ap2(wv, 128, 0, 5, 2, 4), _ap2(pv, 128, o, 5, 2, 4)
            )
            nc.vector.reciprocal_approx_fast(
                _ap2(wv, W4, 4, 5, 2, 1), _ap2(pv, W4, o + 4, 5, 2, 1)
            )
            nc.vector.memset(_ap2(vq_new, 128, o + 4, 5, 2, 1), 0.0)
            nc.vector.tensor_scalar(
                _ap2(vq_new, 128, o, 5, 2, 4), _ap2(wv, 128, 0, 5, 2, 4),
                MU_SCALE / CP1, None, MULT,
            )
            nc.vector.tensor_scalar(
                _ap2(vq_new, W4, o + 4, 5, 2, 1), _ap2(wv, W4, 4, 5, 2, 1),
                MU_SCALE / CP1, None, MULT,
            )

        uqf = None
        for it in range(ITERS):
            last = it == ITERS - 1
            pu = pools["accp"].tile([128, 32], F32, tag="acc")
            pools["pu"] = pu
            uq = pools["vecp"].tile([128, 32], F16, tag="uq")
            if last:
                uqf = pools["vecp"].tile([128, 32], F32, tag="uqf")
            vq_new = pools["vecp"].tile([128, 20], F16, tag="vq")
            # software pipeline: DVE half-ops run under the other half's MMs
            _emit_kv(nc, pools, kts[0], vq, 0)
            _emit_kv(nc, pools, kts[1], vq, 1)
            _emit_kv(nc, pools, kts[2], vq, 2)
            u_half(pu, uq, 0, uqf if last else None)
            _emit_kv(nc, pools, kts[3], vq, 3)
            pv = pools["accp"].tile([128, 20], F32, tag="acc")
            pools["pv"] = pv
            _emit_ktu(nc, pools, kns[0], uq, 0)
            u_half(pu, uq, 1, uqf if last else None)
            _emit_ktu(nc, pools, kns[1], uq, 1)
            _emit_ktu(nc, pools, kns[2], uq, 2)
            v_half(pv, vq_new, 0)
            _emit_ktu(nc, pools, kns[3], uq, 3)
            v_half(pv, vq_new, 1)
            vq = vq_new
        return uqf, vq

    def p_pair(p, s0, built, uq, vq):
        """P = KN * u[n] * v[c]/SC for samples 2p, 2p+1 (quad base s0).
        STT runs on GpSimd (all-SBUF operands) to keep DVE free."""
        for s in (2 * p, 2 * p + 1):
            k = s - s0
            kn = built[k][0]
            ptv = pools["ptp"].tile([128, N], F16, tag="pt")
            for j in range(5):
                w = 128 if j < 4 else W4
                nc.tensor.transpose(
                    ptv[0:1, 128 * j : 128 * j + w],
                    vq[0:w, 5 * k + j : 5 * k + j + 1],
                    sb_ident[0:w, 0:w],
                )
            vsb = pools["vecp"].tile([1, 640], F16, tag="vsb")
            nc.vector.tensor_copy(vsb[:, 0:CP1], ptv[0:1, 0:CP1])
            pr0 = pools["prp"].tile([128, 512], F32, tag="pr")
            pr1 = pools["prp"].tile([128, W4], F32, tag="pr")
            for j in range(5):
                w = 128 if j < 4 else W4
                dst = pr0[:, 128 * j : 128 * j + w] if j < 4 else pr1[:]
                nc.tensor.matmul(
                    dst, lhsT=sb_ones128[:], rhs=vsb[0:1, 128 * j : 128 * j + w],
                    start=True, stop=True,
                )
            vrep = pools["vecp"].tile([128, 560], F16, tag="vrep")
            nc.vector.tensor_copy(vrep[:, 0:512], pr0[:])
            nc.vector.tensor_copy(vrep[:, 512:CP1], pr1[:])
            for t in range(NT):
                tmp = pools["outp"].tile([128, CP1], F16, tag="tmp")
                nc.gpsimd.tensor_tensor(tmp[:], kn[:, t, 0:CP1], vrep[:, 0:CP1], MULT)
                po = pools["outp"].tile([128, CP1], F16, tag="po")
                nc.vector.tensor_scalar(
                    po[:], tmp[:], uq[:, 8 * k + t : 8 * k + t + 1], None, MULT
                )
                nc.sync.dma_start(out[s, 128 * t : 128 * (t + 1), :], po[:])

    # pair-pipelined emission: P-passes of quad 0 interleave with the
    # builds of quad 1 so no engine sits behind a serial tail
    b0 = build_pair(0)
    b1 = build_pair(1)
    uq0, vq0 = iter_quad(0, b0 + b1)
    p_pair(0, 0, b0 + b1, uq0, vq0)
    b2 = build_pair(2)
    p_pair(1, 0, b0 + b1, uq0, vq0)
    b3 = build_pair(3)
    uq1, vq1 = iter_quad(4, b2 + b3)
    p_pair(2, 4, b2 + b3, uq1, vq1)
    p_pair(3, 4, b2 + b3, uq1, vq1)


_NC_CACHE = None


def _get_nc():
    global _NC_CACHE
    if _NC_CACHE is not None:
        return _NC_CACHE
    nc = bacc.Bacc(
        "TRN2", target_bir_lowering=False, debug=False,
        enable_asserts=False, num_devices=NCORES,
    )
    lg = nc.dram_tensor("logits", [S, N, C], F32, kind="ExternalInput").ap()
    mu = nc.dram_tensor("mu", [128, S, NT], F32, kind="ExternalInput").ap()
    gneg = nc.dram_tensor("gneg", [128, S], F32, kind="ExternalInput").ap()
    edg = nc.dram_tensor("edg", [128, S], F32, kind="ExternalInput").ap()
    ident = nc.dram_tensor("ident", [128, 128], F16, kind="ExternalInput").ap()
    out = nc.dram_tensor("out", [S, N, CP1], F16, kind="ExternalOutput").ap()
    with tile.TileContext(nc) as tc, ExitStack() as ctx:
        _build_kernel(ctx, tc, out, lg, mu, gneg, edg, ident)
    nc.compile()
    _NC_CACHE = nc
    return nc


def make_in_maps(logits, visible_mask, dustbin_col_score):
    logits = np.ascontiguousarray(np.asarray(logits, dtype=np.float32))
    mask = np.asarray(visible_mask).astype(bool)
    d = float(np.asarray(dustbin_col_score).reshape(-1)[0])
    g = np.maximum(logits.max(axis=(1, 2)), d).astype(np.float32)      # [B]
    nv = mask.sum(-1).astype(np.float32)
    mu = (MU_SCALE * mask / np.maximum(nv, 1.0)[:, None]).astype(np.float32)
    # column layout per core: mucol[p, s, t] = mu[core*S+s, 128*t+p]
    mucol = np.ascontiguousarray(
        mu.reshape(B, NT, 128).transpose(2, 0, 1)
    ).astype(np.float32)                                               # [128, B, NT]
    gneg = np.repeat(-g[None, :], 128, axis=0).astype(np.float32)      # [128, B]
    edg = np.repeat(np.exp(d - g)[None, :], 128, axis=0).astype(np.float32)
    ident = np.eye(128, dtype=np.float16)
    in_maps = []
    for i in range(NCORES):
        sl = slice(i * S, (i + 1) * S)
        in_maps.append({
            "logits": logits[sl],
            "mu": np.ascontiguousarray(mucol[:, sl, :]),
            "gneg": np.ascontiguousarray(gneg[:, sl]),
            "edg": np.ascontiguousarray(edg[:, sl]),
            "ident": ident,
        })
    return in_maps


def kernel(logits, visible_mask, dustbin_col_score):
    nc = _get_nc()
    in_maps = make_in_maps(logits, visible_mask, dustbin_col_score)
    res = run_bass_kernel_spmd(nc, in_maps, core_ids=list(range(NCORES)))
    P = np.concatenate([res.results[i]["out"] for i in range(NCORES)], axis=0)
    return np.ascontiguousarray(P.astype(np.float32))


# revision 26
# speedup vs baseline: 1.1274x; 1.1274x over previous
"""Sinkhorn AssignmentLoss kernel for 8 TRN2 NeuronCores.

Math: the reference's stabilized log-space Sinkhorn is equivalent (exactly,
up to fp rounding) to exp-space Sinkhorn on the positive kernel matrix
  K2 = [exp(logits - g), rowsum(exp(logits - g)) * exp(d - g)]   # [N, C+1]
with per-sample scalar g = max(max(logits), d) (scale invariance lets us drop
the softmax row-normalization into u):
  u = mu / (K2 v);  v = nu / (K2^T u);  P = diag(u) K2 diag(v)
With TEMP=1 the iteration converges in <4 iterations (measured ~6e-4 rel err
vs the reference's 20 iterations at ITERS=3, fp16 kernel storage).

Per core: 8 samples, data-parallel over batch (no collectives), processed as
four pipelined pairs. The first half-iteration uses the closed form
K2 @ 1 = rowsum(exp) * (1 + exp(d - g)), so the transposed kernel copy is
only needed from iteration 2 onward and its construction overlaps compute.

Device pipeline per sample:
  DMA logits -> ACT exp(+rowsum accum) -> fp16 KN [n-part, c-free],
    zero-padded to 640 cols so every weight chunk is 128 wide (FWL)
  PE transpose -> fp16 KT [c-part, n-free]   (overlapped with iteration 1)
  weights-form matvecs: K chunks are PE weights (fp16 FWL), u/v column
    vectors are the 1-wide moving operand, so matvec results land as PSUM
    columns and reciprocal_approx_fast + multiply run on all 128 DVE lanes.
  P = KN * u[n] * v[c]/SC, tiles split between GpSimd and DVE -> fp16 DMA out
  (host upcasts to fp32 — errors stay ~1e-4 of max|P|)
"""

import sys
import numpy as np

for _p in ("/opt/trn_rl_repo", "/root/.axon_site/_ro/trn_rl_repo"):
    if _p not in sys.path:
        sys.path.insert(0, _p)

from contextlib import ExitStack

import concourse.bass as bass
import concourse.tile as tile
from concourse import bacc, mybir
from concourse.bass_utils import run_bass_kernel_spmd

B, N, C = 64, 1024, 558
CP1 = C + 1
CPAD = 640               # KN free size: 5 chunks of 128
NCORES = 8
S = B // NCORES          # samples per core
NT = N // 128            # 8 row tiles
W4 = CP1 - 512           # 47: logical width of the last c-chunk
ITERS = 3
MU_SCALE = 256.0         # keeps u, v in fp16 normal range; cancels exactly in P

F32 = mybir.dt.float32
F16 = mybir.dt.float16
EXP = mybir.ActivationFunctionType.Exp
MULT = mybir.AluOpType.mult


def _ap2(t, part, off, step, cnt, inner):
    """AP with partitions [0:part], free dims [[step, cnt], [1, inner]]."""
    a = t[:]
    base = list(a.ap)
    return bass.AP(
        tensor=a.tensor,
        offset=a.offset + off * base[-1][0],
        ap=[[base[0][0], part], [step * base[-1][0], cnt], [base[-1][0], inner]],
    )


def _build_kernel(ctx: ExitStack, tc: "tile.TileContext", out, lg, mu, gneg, edg, edg1, ident):
    nc = tc.nc

    pools = {
        "singles": ctx.enter_context(tc.tile_pool(name="singles", bufs=1)),
        "lgp": ctx.enter_context(tc.tile_pool(name="lgp", bufs=6)),
        "knp": ctx.enter_context(tc.tile_pool(name="knp", bufs=4)),
        "ktp": ctx.enter_context(tc.tile_pool(name="ktp", bufs=4)),
        "vecp": ctx.enter_context(tc.tile_pool(name="vecp", bufs=3)),
        "outp": ctx.enter_context(tc.tile_pool(name="outp", bufs=4)),
        "ptp": ctx.enter_context(tc.tile_pool(name="ptp", bufs=2, space="PSUM")),
        "accp": ctx.enter_context(tc.tile_pool(name="accp", bufs=4, space="PSUM")),
        "prp": ctx.enter_context(tc.tile_pool(name="prp", bufs=2, space="PSUM")),
    }
    singles = pools["singles"]

    sb_ident = singles.tile([128, 128], F16)
    nc.sync.dma_start(sb_ident[:], ident)
    sb_gneg = singles.tile([128, S], F32)
    nc.sync.dma_start(sb_gneg[:], gneg)
    sb_edg = singles.tile([128, S], F32)
    nc.sync.dma_start(sb_edg[:], edg)
    sb_edg1 = singles.tile([128, S], F32)
    nc.sync.dma_start(sb_edg1[:], edg1)
    # mu in column layout: mucol[p, s, t] = MU_SCALE * mask/nv at row 128*t+p
    sb_mu = singles.tile([128, S, NT], F32)
    nc.sync.dma_start(sb_mu[:], mu)
    # broadcast weights carry 1/MU_SCALE so P = kn * u' * v'/SC
    sb_ones128 = singles.tile([1, 128], F16)
    nc.vector.memset(sb_ones128[:], 1.0 / MU_SCALE)

    def emit_exp(s):
        """load + exp + rowsums + dustbin + zero pad for one sample."""
        h0 = pools["lgp"].tile([128, 4, C], F32, tag="lgt")
        nc.sync.dma_start(h0[:], lg[s, 0:512].rearrange("(t p) c -> p t c", p=128))
        h1 = pools["lgp"].tile([128, 4, C], F32, tag="lgt")
        nc.sync.dma_start(h1[:], lg[s, 512:1024].rearrange("(t p) c -> p t c", p=128))
        kn = pools["knp"].tile([128, NT, CPAD], F16, tag="kn")
        sacc = pools["vecp"].tile([128, NT], F32, tag="sacc")
        nc.gpsimd.memset(kn[:, :, CP1:CPAD], 0.0)
        for t in range(NT):
            src = h0 if t < 4 else h1
            nc.scalar.activation(
                kn[:, t, 0:C], src[:, t % 4, :], EXP,
                bias=sb_gneg[:, s : s + 1], scale=1.0,
                accum_out=sacc[:, t : t + 1],
            )
        nc.vector.tensor_scalar(
            kn[:, :, C], sacc[:], sb_edg[:, s : s + 1], None, MULT
        )
        return kn, sacc

    def emit_transpose(s, kn):
        kt = pools["ktp"].tile([128, 5, N], F16, tag="kt")
        for j in range(5):
            pt = pools["ptp"].tile([128, N], F16, tag="pt")
            for t in range(NT):
                nc.tensor.transpose(
                    pt[:, 128 * t : 128 * (t + 1)],
                    kn[:, t, 128 * j : 128 * (j + 1)],
                    sb_ident[:],
                )
            if (s + j) % 2 == 0:
                nc.scalar.copy(kt[:, j, :], pt[:])
            else:
                nc.vector.tensor_copy(kt[:, j, :], pt[:])
        return kt

    def emit_kv(kt, vq, k, pu):
        for t in range(NT):
            for j in range(5):
                nc.tensor.matmul(
                    pu[:, 8 * k + t : 8 * k + t + 1],
                    lhsT=kt[:, j, 128 * t : 128 * (t + 1)],
                    rhs=vq[:, 5 * k + j : 5 * k + j + 1],
                    start=(j == 0), stop=(j == 4),
                )

    def emit_ktu(kn, uq, k, pv):
        for j in range(5):
            for t in range(NT):
                nc.tensor.matmul(
                    pv[:, 5 * k + j : 5 * k + j + 1],
                    lhsT=kn[:, t, 128 * j : 128 * (j + 1)],
                    rhs=uq[:, 8 * k + t : 8 * k + t + 1],
                    start=(t == 0), stop=(t == NT - 1),
                )

    def emit_u1(s, k, sacc, uq, uqf):
        """closed-form first u: u1 = mu / (rowsum * (1 + exp(d-g)))."""
        o = 8 * k
        r0 = pools["vecp"].tile([128, NT], F32, tag="r0")
        nc.vector.tensor_scalar(r0[:], sacc[:], sb_edg1[:, s : s + 1], None, MULT)
        wu = pools["vecp"].tile([128, NT], F32, tag="wu")
        nc.vector.reciprocal_approx_fast(wu[:], r0[:])
        mu_sl = sb_mu[:, s, :]
        nc.vector.tensor_mul(uq[:, o : o + 8], mu_sl, wu[:])
        if uqf is not None:
            nc.vector.tensor_mul(uqf[:, o : o + 8], mu_sl, wu[:])

    def emit_u(s, k, pu, uq, uqf):
        o = 8 * k
        wu = pools["vecp"].tile([128, NT], F32, tag="wu")
        nc.vector.reciprocal_approx_fast(wu[:], pu[:, o : o + 8])
        mu_sl = sb_mu[:, s, :]
        nc.vector.tensor_mul(uq[:, o : o + 8], mu_sl, wu[:])
        if uqf is not None:
            nc.vector.tensor_mul(uqf[:, o : o + 8], mu_sl, wu[:])

    def emit_v(k, pv, vq_new):
        o = 5 * k
        wv = pools["vecp"].tile([128, 5], F32, tag="wv")
        nc.vector.reciprocal_approx_fast(wv[:, 0:4], pv[:, o : o + 4])
        nc.vector.reciprocal_approx_fast(wv[0:W4, 4:5], pv[0:W4, o + 4 : o + 5])
        nc.vector.memset(vq_new[:, o + 4 : o + 5], 0.0)
        nc.vector.tensor_scalar(
            vq_new[:, o : o + 4], wv[:, 0:4], MU_SCALE / CP1, None, MULT
        )
        nc.vector.tensor_scalar(
            vq_new[0:W4, o + 4 : o + 5], wv[0:W4, 4:5], MU_SCALE / CP1, None, MULT
        )

    def emit_p(s, k, kn, uqf, vq):
        """P = KN * u[n] * v[c]/SC; tiles alternate GpSimd / DVE."""
        ptv = pools["ptp"].tile([128, N], F16, tag="pt")
        for j in range(5):
            w = 128 if j < 4 else W4
            nc.tensor.transpose(
                ptv[0:1, 128 * j : 128 * j + w],
                vq[0:w, 5 * k + j : 5 * k + j + 1],
                sb_ident[0:w, 0:w],
            )
        vsb = pools["vecp"].tile([1, 640], F16, tag="vsb")
        nc.vector.tensor_copy(vsb[:, 0:CP1], ptv[0:1, 0:CP1])
        pr0 = pools["prp"].tile([128, 512], F32, tag="pr")
        pr1 = pools["prp"].tile([128, W4], F32, tag="pr")
        for j in range(5):
            w = 128 if j < 4 else W4
            dst = pr0[:, 128 * j : 128 * j + w] if j < 4 else pr1[:]
            nc.tensor.matmul(
                dst, lhsT=sb_ones128[:], rhs=vsb[0:1, 128 * j : 128 * j + w],
                start=True, stop=True,
            )
        vrep = pools["vecp"].tile([128, 560], F16, tag="vrep")
        nc.vector.tensor_copy(vrep[:, 0:512], pr0[:])
        nc.vector.tensor_copy(vrep[:, 512:CP1], pr1[:])
        ucol = lambda t: uqf[:, 8 * k + t : 8 * k + t + 1]
        for t in range(NT):
            po = pools["outp"].tile([128, CP1], F16, tag="po")
            if t % 2 == 0:
                tmp = pools["outp"].tile([128, CP1], F16, tag="tmp")
                nc.gpsimd.tensor_tensor(
                    tmp[:], kn[:, t, 0:CP1], vrep[:, 0:CP1], MULT
                )
                nc.vector.tensor_scalar(po[:], tmp[:], ucol(t), None, MULT)
            else:
                nc.vector.scalar_tensor_tensor(
                    po[:], kn[:, t, 0:CP1], ucol(t), vrep[:, 0:CP1], MULT, MULT
                )
            nc.sync.dma_start(out[s, 128 * t : 128 * (t + 1), :], po[:])

    for p in range(S // 2):
        sA, sB = 2 * p, 2 * p + 1
        knA, saccA = emit_exp(sA)
        knB, saccB = emit_exp(sB)
        vq = pools["vecp"].tile([128, 10], F16, tag="vq")
        uq = pools["vecp"].tile([128, 16], F16, tag="uq")
        uqf = None
        if ITERS == 1:
            uqf = pools["vecp"].tile([128, 16], F32, tag="uqf")
        # iteration 1: closed-form Kv, then K^T u on KN only
        emit_u1(sA, 0, saccA, uq, uqf)
        emit_u1(sB, 1, saccB, uq, uqf)
        pv = pools["accp"].tile([128, 10], F32, tag="acc")
        emit_ktu(knA, uq, 0, pv)
        emit_ktu(knB, uq, 1, pv)
        emit_v(0, pv, vq)
        # transposes overlap iteration 1 on the PE stream
        ktA = emit_transpose(sA, knA)
        emit_v(1, pv, vq)
        ktB = emit_transpose(sB, knB)
        for it in range(1, ITERS):
            last = it == ITERS - 1
            pu = pools["accp"].tile([128, 16], F32, tag="acc")
            uq = pools["vecp"].tile([128, 16], F16, tag="uq")
            if last:
                uqf = pools["vecp"].tile([128, 16], F32, tag="uqf")
            emit_kv(ktA, vq, 0, pu)
            emit_kv(ktB, vq, 1, pu)
            emit_u(sA, 0, pu, uq, uqf if last else None)
            pv = pools["accp"].tile([128, 10], F32, tag="acc")
            emit_ktu(knA, uq, 0, pv)
            emit_u(sB, 1, pu, uq, uqf if last else None)
            emit_ktu(knB, uq, 1, pv)
            vq_new = pools["vecp"].tile([128, 10], F16, tag="vq")
            emit_v(0, pv, vq_new)
            emit_v(1, pv, vq_new)
            vq = vq_new
        emit_p(sA, 0, knA, uqf, vq)
        emit_p(sB, 1, knB, uqf, vq)


_NC_CACHE = None


def _get_nc():
    global _NC_CACHE
    if _NC_CACHE is not None:
        return _NC_CACHE
    nc = bacc.Bacc(
        "TRN2", target_bir_lowering=False, debug=False,
        enable_asserts=False, num_devices=NCORES,
    )
    lg = nc.dram_tensor("logits", [S, N, C], F32, kind="ExternalInput").ap()
    mu = nc.dram_tensor("mu", [128, S, NT], F32, kind="ExternalInput").ap()
    gneg = nc.dram_tensor("gneg", [128, S], F32, kind="ExternalInput").ap()
    edg = nc.dram_tensor("edg", [128, S], F32, kind="ExternalInput").ap()
    edg1 = nc.dram_tensor("edg1", [128, S], F32, kind="ExternalInput").ap()
    ident = nc.dram_tensor("ident", [128, 128], F16, kind="ExternalInput").ap()
    out = nc.dram_tensor("out", [S, N, CP1], F16, kind="ExternalOutput").ap()
    with tile.TileContext(nc) as tc, ExitStack() as ctx:
        _build_kernel(ctx, tc, out, lg, mu, gneg, edg, edg1, ident)
    nc.compile()
    _NC_CACHE = nc
    return nc


def make_in_maps(logits, visible_mask, dustbin_col_score):
    logits = np.ascontiguousarray(np.asarray(logits, dtype=np.float32))
    mask = np.asarray(visible_mask).astype(bool)
    d = float(np.asarray(dustbin_col_score).reshape(-1)[0])
    g = np.maximum(logits.max(axis=(1, 2)), d).astype(np.float32)      # [B]
    nv = mask.sum(-1).astype(np.float32)
    mu = (MU_SCALE * mask / np.maximum(nv, 1.0)[:, None]).astype(np.float32)
    # column layout per core: mucol[p, s, t] = mu[core*S+s, 128*t+p]
    mucol = np.ascontiguousarray(
        mu.reshape(B, NT, 128).transpose(2, 0, 1)
    ).astype(np.float32)                                               # [128, B, NT]
    gneg = np.repeat(-g[None, :], 128, axis=0).astype(np.float32)      # [128, B]
    edgv = np.exp(d - g).astype(np.float32)
    edg = np.repeat(edgv[None, :], 128, axis=0).astype(np.float32)
    edg1 = np.repeat((1.0 + edgv)[None, :], 128, axis=0).astype(np.float32)
    ident = np.eye(128, dtype=np.float16)
    in_maps = []
    for i in range(NCORES):
        sl = slice(i * S, (i + 1) * S)
        in_maps.append({
            "logits": logits[sl],
            "mu": np.ascontiguousarray(mucol[:, sl, :]),
            "gneg": np.ascontiguousarray(gneg[:, sl]),
            "edg": np.ascontiguousarray(edg[:, sl]),
            "edg1": np.ascontiguousarray(edg1[:, sl]),
            "ident": ident,
        })
    return in_maps


def kernel(logits, visible_mask, dustbin_col_score):
    nc = _get_nc()
    in_maps = make_in_maps(logits, visible_mask, dustbin_col_score)
    res = run_bass_kernel_spmd(nc, in_maps, core_ids=list(range(NCORES)))
    P = np.concatenate([res.results[i]["out"] for i in range(NCORES)], axis=0)
    return np.ascontiguousarray(P.astype(np.float32))
